# revision 31
# baseline (speedup 1.0000x reference)
import os
import sys
import numpy as np

try:
    import concourse  # noqa: F401
except ImportError:
    for _p in ("/opt/trn_rl_repo", "/root/.axon_site/_ro/trn_rl_repo"):
        if os.path.isdir(_p):
            sys.path.insert(0, _p)
            break

from concourse import bass, bacc, tile, mybir, bass_utils  # noqa: E402

# Problem constants (nn_DefectPredictionGNN: 3-layer GCN + mean-pool + 3 heads)
N, E, B = 50000, 800000, 64
IN, H = 3, 64
NDEF = 6
EPS = 1e-5
NCORE = 8
NB = 52                      # dst blocks per core
SHARD = NB * 128             # 6656 padded slots per core
LOROWS = 4 * SHARD           # 26624 (< int16 range)
BLK_PER_SEG = 5              # blocks per gather segment
NSEG = NB // BLK_PER_SEG     # 10

_cache = {}
_last_cfg = [(9, 9)]


def _plan(src, dst, batch):
    """Host-side index preprocessing: node->core/block/slot assignment,
    per-core gather lists and chunk metadata. Pure integer work."""
    deg = np.bincount(dst, minlength=N).astype(np.int64)   # in-degree (no self)
    degp1 = deg + 1

    # Stage 1: assign nodes to cores, snake over degree-sorted order.
    order = np.argsort(-degp1, kind="stable")
    core_of = np.empty(N, np.int32)
    snake = np.concatenate([np.arange(NCORE), np.arange(NCORE)[::-1]])
    core_of[order] = snake[np.arange(N) % (2 * NCORE)]

    # Self-loops are handled on-device via a diagonal scaled-shot matmul
    # (ms is already resident in SBUF) -- they do not enter the gather lists.
    if os.environ.get("K_SELF_EDGES") == "1":
        src_all = np.concatenate([src, np.arange(N, dtype=src.dtype)])
        dst_all = np.concatenate([dst, np.arange(N, dtype=dst.dtype)])
    else:
        src_all, dst_all = src, dst

    # lo/hi membership of a SOURCE node = its core's half (cores 0-3 are rows
    # [0, LOROWS)). Known after stage 1, independent of block assignment.
    lodeg = np.bincount(dst_all[core_of[src_all] < 4], minlength=N)
    hideg = np.bincount(dst_all[core_of[src_all] >= 4], minlength=N)

    # Stage 2: per core, greedily pack nodes into 49 blocks (<=128 nodes each)
    # balancing (lo, hi) in-edge loads.
    block_of = np.empty(N, np.int32)
    slot_of = np.empty(N, np.int32)
    for c in range(NCORE):
        nodes = np.where(core_of == c)[0]
        nodes = nodes[np.argsort(-(lodeg[nodes] + hideg[nodes]), kind="stable")]
        loads_lo = np.zeros(NB, np.int64)
        loads_hi = np.zeros(NB, np.int64)
        counts = np.zeros(NB, np.int64)
        for u in nodes:
            lv, hv = lodeg[u], hideg[u]
            cand = np.maximum(loads_lo + lv, loads_hi + hv).astype(np.float64)
            cand[counts >= 128] = np.inf
            j = int(np.argmin(cand))
            block_of[u] = j
            slot_of[u] = counts[j]
            counts[j] += 1
            loads_lo[j] += lv
            loads_hi[j] += hv

    # Global HBM row of each node: (core, block, slot)
    row_of = (core_of.astype(np.int64) * SHARD + block_of * 128 + slot_of)

    # Edge placement
    e_core = core_of[dst_all]
    e_block = block_of[dst_all]
    e_slot = slot_of[dst_all]
    e_srow = row_of[src_all]
    e_islo = e_srow < LOROWS
    e_deg = degp1[dst_all]

    # Chunk capacity per (core, block, half)
    max_lo = max_hi = 0
    per = {}
    for c in range(NCORE):
        mc = e_core == c
        for half, mh in (("lo", e_islo), ("hi", ~e_islo)):
            m = mc & mh
            cnt = np.bincount(e_block[m], minlength=NB)
            per[(c, half)] = m
            if half == "lo":
                max_lo = max(max_lo, int(cnt.max()))
            else:
                max_hi = max(max_hi, int(cnt.max()))
    c_lo = max(1, -(-max_lo // 128))
    c_hi = max(1, -(-max_hi // 128))
    nchunk = NB * (c_lo + c_hi)

    plans = []
    for c in range(NCORE):
        idx_lo = np.zeros((NB, c_lo * 128), np.int16)
        idx_hi = np.zeros((NB, c_hi * 128), np.int16)
        dstloc = np.full((nchunk, 128), -1.0, np.float32)
        degedge = np.ones((nchunk, 128), np.float32)
        for half, idx_arr, cc, off in (("lo", idx_lo, c_lo, 0), ("hi", idx_hi, c_hi, c_lo)):
            m = (e_core == c) & (e_islo if half == "lo" else ~e_islo)
            eb, es, er, ed = e_block[m], e_slot[m], e_srow[m], e_deg[m]
            if half == "hi":
                er = er - LOROWS
            o = np.argsort(eb, kind="stable")
            eb, es, er, ed = eb[o], es[o], er[o], ed[o]
            starts = np.searchsorted(eb, np.arange(NB))
            ends = np.searchsorted(eb, np.arange(NB) + 1)
            for b in range(NB):
                k = ends[b] - starts[b]
                sl = slice(starts[b], ends[b])
                idx_arr[b, :k] = er[sl].astype(np.int16)
                gc0 = b * (c_lo + c_hi) + off
                dl = dstloc[gc0:gc0 + cc].reshape(-1)
                dl[:k] = es[sl]
                de = degedge[gc0:gc0 + cc].reshape(-1)
                de[:k] = ed[sl]

        def wrap(a):  # [L] int16 -> [128, L//16] wrapped + replicated
            L = a.size
            w = a.reshape(L // 16, 16).T  # [16, L//16]
            return np.tile(w, (8, 1)).copy()

        degown = np.ones((NB, 128), np.float32)
        mine = np.where(core_of == c)[0]
        degown[block_of[mine], slot_of[mine]] = degp1[mine]

        bsc = np.zeros((NB, 128, B), np.float32)
        cnt = np.bincount(batch, minlength=B).astype(np.float32)
        invcnt = 1.0 / np.maximum(cnt, 1.0)
        bsc[block_of[mine], slot_of[mine], batch[mine]] = invcnt[batch[mine]]

        plans.append(dict(
            idxlo=wrap(idx_lo.reshape(-1)),
            idxhi=wrap(idx_hi.reshape(-1)),
            dstloc=np.ascontiguousarray(dstloc.T),
            degedge=np.ascontiguousarray(degedge.T),
            degown=np.ascontiguousarray(degown.reshape(NB * 128)
                                        .reshape(NB, 128).T),  # [128, NB]
            bsc=np.ascontiguousarray(bsc.transpose(1, 0, 2)),  # [128, NB, B]
            mine=mine,
        ))
    return dict(plans=plans, c_lo=c_lo, c_hi=c_hi, nchunk=nchunk,
                core_of=core_of, block_of=block_of, slot_of=slot_of,
                row_of=row_of)


def _build(c_lo, c_hi):
    nchunk = NB * (c_lo + c_hi)
    f32 = mybir.dt.float32
    nc = bacc.Bacc("TRN2", target_bir_lowering=False, debug=False,
                   num_devices=NCORE)

    def di(name, shape, dt=f32):
        return nc.dram_tensor(name, shape, dt, kind="ExternalInput")

    bf16 = mybir.dt.bfloat16
    xt = di("xt", [IN, SHARD])
    idxlo = di("idxlo", [128, NB * c_lo * 8], mybir.dt.int16)
    idxhi = di("idxhi", [128, NB * c_hi * 8], mybir.dt.int16)
    dstloc = di("dstloc", [128, nchunk])
    degedge = di("degedge", [128, nchunk])
    degown = di("degown", [128, NB])
    iota = di("iota", [128, 128], bf16)
    slotid = di("slotid", [128, 1])
    ident = di("ident", [128, 128])
    bsc = di("bsc", [128, NB, B])
    ws = [di(f"w{l}t", [IN if l == 1 else H, H]) for l in (1, 2, 3)]
    tbs = [di(f"tb{l}", [H, 1]) for l in (1, 2, 3)]
    hw = {}
    for hname, h1 in (("th", 32), ("lh", 32), ("sh", 16)):
        h2 = {"th": NDEF, "lh": 2, "sh": 1}[hname]
        hw[hname] = (di(f"{hname}w1", [H, h1]), di(f"{hname}b1", [h1, 1]),
                     di(f"{hname}w2", [h1, h2]), di(f"{hname}b2", [h2, 1]))
    out = nc.dram_tensor("out", [NDEF + 3, B], f32, kind="ExternalOutput")

    AF = mybir.ActivationFunctionType
    OP = mybir.AluOpType
    # segment sizes in blocks (ragged: ramp up, then steady)
    STEADY = BLK_PER_SEG
    sizes = []
    for sz in (1, 1, 2, 3, 5):
        if sum(sizes) + sz <= NB:
            sizes.append(sz)
    while sum(sizes) < NB:
        sizes.append(min(STEADY, NB - sum(sizes)))
    SEGS = []
    acc = 0
    for sz in sizes:
        SEGS.append((acc, sz))
        acc += sz
    SEG_LO, SEG_HI = max(sz for _, sz in SEGS) * c_lo, max(sz for _, sz in SEGS) * c_hi

    with tile.TileContext(nc) as tc:
        with (
            tc.tile_pool(name="const", bufs=1) as cpool,
            tc.tile_pool(name="g", bufs=3) as gpool,
            tc.tile_pool(name="ht", bufs=2 * NB) as hpool,
            tc.tile_pool(name="work", bufs=12) as wpool,
            tc.tile_pool(name="ms", bufs=NB + 12) as mpool,
            tc.tile_pool(name="psA", bufs=2, space="PSUM") as psA,
            tc.tile_pool(name="psB", bufs=3, space="PSUM") as psB,
            tc.tile_pool(name="psAcc", bufs=1, space="PSUM") as psAcc,
            tc.tile_pool(name="psC", bufs=2, space="PSUM") as psC,
            tc.tile_pool(name="dram", bufs=1, space="DRAM") as dpool,
        ):
            def load(dram_t, shape, dt=f32, tag=None):
                t = cpool.tile(shape, dt, tag=tag or dram_t.name)
                nc.sync.dma_start(t[:], dram_t[:])
                return t

            xt_sb = load(xt, [IN, SHARD])
            ilo_sb = load(idxlo, [128, NB * c_lo * 8], mybir.dt.int16)
            ihi_sb = load(idxhi, [128, NB * c_hi * 8], mybir.dt.int16)
            dl_sb = load(dstloc, [128, nchunk])
            de_sb = load(degedge, [128, nchunk])
            down_sb = load(degown, [128, NB])
            iota_sb = load(iota, [128, 128], bf16)
            sid_sb = load(slotid, [128, 1])
            id_sb = load(ident, [128, 128])
            bsc_sb = load(bsc, [128, NB, B])
            w_sb = [load(w, [IN if l == 1 else H, H]) for l, w in zip((1, 2, 3), ws)]
            tb_sb = [load(t, [H, 1]) for t in tbs]
            hw_sb = {k: tuple(load(t, list(t.shape), tag=f"{k}{i}")
                              for i, t in enumerate(v)) for k, v in hw.items()}

            # dis = 1/sqrt(deg+1)
            dise = cpool.tile([128, nchunk], f32, tag="dise")
            nc.vector.reciprocal(dise[:], de_sb[:])
            nc.scalar.activation(dise[:], dise[:], AF.Sqrt)

            diso = cpool.tile([128, NB], f32, tag="diso")
            nc.vector.reciprocal(diso[:], down_sb[:])
            nc.scalar.activation(diso[:], diso[:], AF.Sqrt)

            bounce = dpool.tile([SHARD, 2 * H], bf16, tag="bounce")
            msA = dpool.tile([NCORE * SHARD, 2 * H], bf16, tag="msA")
            msB = dpool.tile([NCORE * SHARD, 2 * H], bf16, tag="msB")
            poolin = dpool.tile([H, B], f32, tag="poolin")
            poolout = dpool.tile([H, B], f32, tag="poolout")

            hT = [None] * NB
            msT = [None] * NB
            for l in (1, 2, 3):
                msfull = (msA, msB, msA)[l - 1]
                # PRE: ms shard = dis * (h @ W~), node-major -> bounce
                for b in range(NB):
                    lhsT = xt_sb[:, b * 128:(b + 1) * 128] if l == 1 else hT[b][:]
                    ps = psA.tile([128, H], f32, tag="pre")
                    nc.tensor.matmul(ps[:], lhsT, w_sb[l - 1][:],
                                     start=True, stop=True)
                    ms = mpool.tile([128, H], bf16, tag="ms")
                    nc.scalar.activation(ms[:], ps[:], AF.Copy,
                                         scale=diso[:, b:b + 1])
                    nc.sync.dma_start(bounce[b * 128:(b + 1) * 128, 0:H], ms[:])
                    msT[b] = ms
                if os.environ.get("K_NO_COLL") == "1":
                    for _r in range(NCORE):
                        nc.sync.dma_start(
                            msfull[_r * SHARD:(_r + 1) * SHARD, :], bounce[:])
                else:
                    nc.gpsimd.collective_compute(
                        "AllGather", OP.bypass,
                        ins=[bounce[:].opt()], outs=[msfull[:].opt()],
                        replica_groups=[list(range(NCORE))])
                # MP: gather + scaled-onehot scatter matmuls.
                # Ragged segments: small first gathers shorten the
                # post-AllGather pipeline bubble.
                for s0, scnt in SEGS:
                    glo = gpool.tile([128, SEG_LO, 2 * H], bf16, tag="glo")
                    ghi = gpool.tile([128, SEG_HI, 2 * H], bf16, tag="ghi")
                    if os.environ.get("K_NO_GATHER") == "1":
                        nc.vector.memset(glo[:], 0.0)
                        nc.vector.memset(ghi[:], 0.0)
                    else:
                        nc.gpsimd.dma_gather(
                            glo[:, 0:scnt * c_lo, :], msfull[0:LOROWS, :],
                            ilo_sb[:, s0 * c_lo * 8:(s0 + scnt) * c_lo * 8],
                            scnt * c_lo * 128, scnt * c_lo * 128, 2 * H,
                            single_packet=False)
                        nc.gpsimd.dma_gather(
                            ghi[:, 0:scnt * c_hi, :], msfull[LOROWS:2 * LOROWS, :],
                            ihi_sb[:, s0 * c_hi * 8:(s0 + scnt) * c_hi * 8],
                            scnt * c_hi * 128, scnt * c_hi * 128, 2 * H,
                            single_packet=False)
                    for bb in range(scnt):
                        b = s0 + bb
                        ps = psB.tile([H, 128], f32, tag="mp")
                        for c in range(c_lo + c_hi):
                            gc = b * (c_lo + c_hi) + c
                            g = (glo[:, bb * c_lo + c, 0:H] if c < c_lo
                                 else ghi[:, bb * c_hi + (c - c_lo), 0:H])
                            shot = wpool.tile([128, 128], bf16, tag="shot")
                            nc.vector.tensor_scalar(
                                shot[:], iota_sb[:],
                                dl_sb[:, gc:gc + 1], dise[:, gc:gc + 1],
                                op0=OP.is_equal, op1=OP.mult)
                            nc.tensor.matmul(ps[:], g, shot[:],
                                             start=(c == 0), stop=False)
                        # self-loop term dis_d * ms_d via diagonal scaled-shot
                        dshot = wpool.tile([128, 128], bf16, tag="shot")
                        nc.vector.tensor_scalar(
                            dshot[:], iota_sb[:],
                            sid_sb[:, 0:1], diso[:, b:b + 1],
                            op0=OP.is_equal, op1=OP.mult)
                        nc.tensor.matmul(ps[:], msT[b][:], dshot[:],
                                         start=False, stop=True)
                        h = hpool.tile([H, 128], f32, tag="hT")
                        nc.scalar.activation(h[:], ps[:], AF.Relu,
                                             bias=tb_sb[l - 1][:, 0:1])
                        hT[b] = h

            # Pooling: gembT = sum_b h3_b^T-free... pooledT[f,g] via transpose
            poolps = psAcc.tile([H, B], f32, tag="poolacc")
            for b in range(NB):
                pst = psC.tile([128, H], f32, tag="scratch")
                nc.tensor.transpose(pst[:], hT[b][:], id_sb[0:H, 0:H])
                h3 = wpool.tile([128, H], f32, tag="h3")
                nc.scalar.activation(h3[:], pst[:], AF.Copy)
                nc.tensor.matmul(poolps[:], h3[:], bsc_sb[:, b, :],
                                 start=(b == 0), stop=(b == NB - 1))
            psb = wpool.tile([H, B], f32, tag="poolsb")
            nc.scalar.activation(psb[:], poolps[:], AF.Copy)
            nc.sync.dma_start(poolin[:], psb[:])
            if os.environ.get("K_NO_COLL") == "1":
                nc.sync.dma_start(poolout[:], poolin[:])
            else:
                nc.gpsimd.collective_compute(
                    "AllReduce", OP.add,
                    ins=[poolin[:].opt()], outs=[poolout[:].opt()],
                    replica_groups=[list(range(NCORE))])
            gemb = wpool.tile([H, B], f32, tag="gemb")
            nc.sync.dma_start(gemb[:], poolout[:])

            # Heads (computed replicated on every core)
            for hname, r0, act in (("th", 0, None), ("lh", NDEF, AF.Sigmoid),
                                   ("sh", NDEF + 2, AF.Sigmoid)):
                w1, b1, w2, b2 = hw_sb[hname]
                h1 = w1.shape[1]
                h2 = w2.shape[1]
                p1t = psC.tile([128, B], f32, tag="scratch")
                p1 = p1t[0:h1, :]
                nc.tensor.matmul(p1, w1[:], gemb[:], start=True, stop=True)
                a1 = wpool.tile([h1, B], f32, tag="hd1sb")
                nc.scalar.activation(a1[:], p1, AF.Relu, bias=b1[:, 0:1])
                p2t = psC.tile([128, B], f32, tag="scratch")
                p2 = p2t[0:h2, :]
                nc.tensor.matmul(p2, w2[:], a1[:], start=True, stop=True)
                hsb = wpool.tile([h2, B], f32, tag="hdout")
                if act is None:
                    nc.vector.tensor_scalar_add(hsb[:], p2, b2[:, 0:1])
                else:
                    nc.scalar.activation(hsb[:], p2, act, bias=b2[:, 0:1])
                nc.sync.dma_start(out[r0:r0 + h2, :], hsb[:])

    nc.compile()
    return nc


def prepare(x, edge_index, batch,
            W1, b1, W2, b2, W3, b3,
            bn1_g, bn1_b, bn1_m, bn1_v,
            bn2_g, bn2_b, bn2_m, bn2_v,
            bn3_g, bn3_b, bn3_m, bn3_v,
            th_W1, th_b1, th_W2, th_b2,
            lh_W1, lh_b1, lh_W2, lh_b2,
            sh_W1, sh_b1, sh_W2, sh_b2):
    x = np.asarray(x, np.float32)
    edge_index = np.asarray(edge_index)
    batch = np.asarray(batch)
    src, dst = np.asarray(edge_index[0], np.int64), np.asarray(edge_index[1], np.int64)

    plan = _plan(src, dst, np.asarray(batch, np.int64))
    c_lo, c_hi = plan["c_lo"], plan["c_hi"]

    key = (c_lo, c_hi)
    _last_cfg[0] = key
    if key not in _cache:
        _cache[key] = _build(c_lo, c_hi)
    nc = _cache[key]

    # BN-folded weights
    def fold(W, bb, g, beta, mu, v):
        s = np.asarray(g) / np.sqrt(np.asarray(v) + EPS)
        Wt = np.asarray(W, np.float32) * s[None, :]
        tb = ((np.asarray(bb) - np.asarray(mu)) * s + np.asarray(beta))
        return Wt.astype(np.float32), tb.astype(np.float32).reshape(H, 1)

    w1t, tb1 = fold(W1, b1, bn1_g, bn1_b, bn1_m, bn1_v)
    w2t, tb2 = fold(W2, b2, bn2_g, bn2_b, bn2_m, bn2_v)
    w3t, tb3 = fold(W3, b3, bn3_g, bn3_b, bn3_m, bn3_v)

    import ml_dtypes
    iota_np = np.tile(np.arange(128, dtype=np.float32), (128, 1)).astype(ml_dtypes.bfloat16)
    ident_np = np.eye(128, dtype=np.float32)

    in_maps = []
    for c in range(NCORE):
        p = plan["plans"][c]
        mine = p["mine"]
        xts = np.zeros((IN, SHARD), np.float32)
        cols = plan["block_of"][mine] * 128 + plan["slot_of"][mine]
        xts[:, cols] = x[mine].T
        in_maps.append({
            "xt": xts, "idxlo": p["idxlo"], "idxhi": p["idxhi"],
            "slotid": np.arange(128, dtype=np.float32).reshape(128, 1),
            "dstloc": p["dstloc"], "degedge": p["degedge"],
            "degown": p["degown"], "iota": iota_np, "ident": ident_np,
            "bsc": p["bsc"],
            "w1t": w1t, "w2t": w2t, "w3t": w3t,
            "tb1": tb1, "tb2": tb2, "tb3": tb3,
            "thw1": np.asarray(th_W1, np.float32), "thb1": np.asarray(th_b1, np.float32).reshape(-1, 1),
            "thw2": np.asarray(th_W2, np.float32), "thb2": np.asarray(th_b2, np.float32).reshape(-1, 1),
            "lhw1": np.asarray(lh_W1, np.float32), "lhb1": np.asarray(lh_b1, np.float32).reshape(-1, 1),
            "lhw2": np.asarray(lh_W2, np.float32), "lhb2": np.asarray(lh_b2, np.float32).reshape(-1, 1),
            "shw1": np.asarray(sh_W1, np.float32), "shb1": np.asarray(sh_b1, np.float32).reshape(-1, 1),
            "shw2": np.asarray(sh_W2, np.float32), "shb2": np.asarray(sh_b2, np.float32).reshape(-1, 1),
        })

    return nc, in_maps


def kernel(**inputs):
    nc, in_maps = prepare(**inputs)
    kernel._last_clo, kernel._last_chi = _last_cfg[0]
    res = bass_utils.run_bass_kernel_spmd(nc, in_maps, core_ids=list(range(NCORE)))
    kernel._last_results = res
    o = res.results[0]["out"]  # [9, B]
    type_logits = np.ascontiguousarray(o[0:NDEF].T)
    location = np.ascontiguousarray(o[NDEF:NDEF + 2].T)
    severity = np.ascontiguousarray(o[NDEF + 2:NDEF + 3].T)
    return (type_logits, location, severity)


# revision 34
# speedup vs baseline: 1.0026x; 1.0026x over previous
import os
import sys
import numpy as np

try:
    import concourse  # noqa: F401
except ImportError:
    for _p in ("/opt/trn_rl_repo", "/root/.axon_site/_ro/trn_rl_repo"):
        if os.path.isdir(_p):
            sys.path.insert(0, _p)
            break

from concourse import bass, bacc, tile, mybir, bass_utils  # noqa: E402

# Problem constants (nn_DefectPredictionGNN: 3-layer GCN + mean-pool + 3 heads)
N, E, B = 50000, 800000, 64
IN, H = 3, 64
NDEF = 6
EPS = 1e-5
NCORE = 8
NB = 52                      # dst blocks per core
SHARD = NB * 128             # 6656 padded slots per core
LOROWS = 4 * SHARD           # 26624 (< int16 range)
BLK_PER_SEG = 5              # blocks per gather segment
NSEG = NB // BLK_PER_SEG     # 10

_cache = {}
_last_cfg = [(9, 9)]


def _plan(src, dst, batch):
    """Host-side index preprocessing: node->core/block/slot assignment,
    per-core gather lists and chunk metadata. Pure integer work."""
    deg = np.bincount(dst, minlength=N).astype(np.int64)   # in-degree (no self)
    degp1 = deg + 1

    # Stage 1: assign nodes to cores, snake over degree-sorted order.
    order = np.argsort(-degp1, kind="stable")
    core_of = np.empty(N, np.int32)
    snake = np.concatenate([np.arange(NCORE), np.arange(NCORE)[::-1]])
    core_of[order] = snake[np.arange(N) % (2 * NCORE)]

    # Self-loops are handled on-device via a diagonal scaled-shot matmul
    # (ms is already resident in SBUF) -- they do not enter the gather lists.
    if os.environ.get("K_SELF_EDGES") == "1":
        src_all = np.concatenate([src, np.arange(N, dtype=src.dtype)])
        dst_all = np.concatenate([dst, np.arange(N, dtype=dst.dtype)])
    else:
        src_all, dst_all = src, dst

    # lo/hi membership of a SOURCE node = its core's half (cores 0-3 are rows
    # [0, LOROWS)). Known after stage 1, independent of block assignment.
    lodeg = np.bincount(dst_all[core_of[src_all] < 4], minlength=N)
    hideg = np.bincount(dst_all[core_of[src_all] >= 4], minlength=N)

    # Stage 2: per core, greedily pack nodes into 49 blocks (<=128 nodes each)
    # balancing (lo, hi) in-edge loads.
    block_of = np.empty(N, np.int32)
    slot_of = np.empty(N, np.int32)
    for c in range(NCORE):
        nodes = np.where(core_of == c)[0]
        nodes = nodes[np.argsort(-(lodeg[nodes] + hideg[nodes]), kind="stable")]
        loads_lo = np.zeros(NB, np.int64)
        loads_hi = np.zeros(NB, np.int64)
        counts = np.zeros(NB, np.int64)
        for u in nodes:
            lv, hv = lodeg[u], hideg[u]
            cand = np.maximum(loads_lo + lv, loads_hi + hv).astype(np.float64)
            cand[counts >= 128] = np.inf
            j = int(np.argmin(cand))
            block_of[u] = j
            slot_of[u] = counts[j]
            counts[j] += 1
            loads_lo[j] += lv
            loads_hi[j] += hv

    # Global HBM row of each node: (core, block, slot)
    row_of = (core_of.astype(np.int64) * SHARD + block_of * 128 + slot_of)

    # Edge placement
    e_core = core_of[dst_all]
    e_block = block_of[dst_all]
    e_slot = slot_of[dst_all]
    e_srow = row_of[src_all]
    e_islo = e_srow < LOROWS
    e_deg = degp1[dst_all]

    # Chunk capacity per (core, block, half)
    max_lo = max_hi = 0
    per = {}
    for c in range(NCORE):
        mc = e_core == c
        for half, mh in (("lo", e_islo), ("hi", ~e_islo)):
            m = mc & mh
            cnt = np.bincount(e_block[m], minlength=NB)
            per[(c, half)] = m
            if half == "lo":
                max_lo = max(max_lo, int(cnt.max()))
            else:
                max_hi = max(max_hi, int(cnt.max()))
    c_lo = max(1, -(-max_lo // 128))
    c_hi = max(1, -(-max_hi // 128))
    nchunk = NB * (c_lo + c_hi)

    plans = []
    for c in range(NCORE):
        idx_lo = np.zeros((NB, c_lo * 128), np.int16)
        idx_hi = np.zeros((NB, c_hi * 128), np.int16)
        dstloc = np.full((nchunk, 128), -1.0, np.float32)
        degedge = np.ones((nchunk, 128), np.float32)
        for half, idx_arr, cc, off in (("lo", idx_lo, c_lo, 0), ("hi", idx_hi, c_hi, c_lo)):
            m = (e_core == c) & (e_islo if half == "lo" else ~e_islo)
            eb, es, er, ed = e_block[m], e_slot[m], e_srow[m], e_deg[m]
            if half == "hi":
                er = er - LOROWS
            o = np.argsort(eb, kind="stable")
            eb, es, er, ed = eb[o], es[o], er[o], ed[o]
            starts = np.searchsorted(eb, np.arange(NB))
            ends = np.searchsorted(eb, np.arange(NB) + 1)
            for b in range(NB):
                k = ends[b] - starts[b]
                sl = slice(starts[b], ends[b])
                idx_arr[b, :k] = er[sl].astype(np.int16)
                gc0 = b * (c_lo + c_hi) + off
                dl = dstloc[gc0:gc0 + cc].reshape(-1)
                dl[:k] = es[sl]
                de = degedge[gc0:gc0 + cc].reshape(-1)
                de[:k] = ed[sl]

        def wrap(a):  # [L] int16 -> [128, L//16] wrapped + replicated
            L = a.size
            w = a.reshape(L // 16, 16).T  # [16, L//16]
            return np.tile(w, (8, 1)).copy()

        degown = np.ones((NB, 128), np.float32)
        mine = np.where(core_of == c)[0]
        degown[block_of[mine], slot_of[mine]] = degp1[mine]

        bsc = np.zeros((NB, 128, B), np.float32)
        cnt = np.bincount(batch, minlength=B).astype(np.float32)
        invcnt = 1.0 / np.maximum(cnt, 1.0)
        bsc[block_of[mine], slot_of[mine], batch[mine]] = invcnt[batch[mine]]

        plans.append(dict(
            idxlo=wrap(idx_lo.reshape(-1)),
            idxhi=wrap(idx_hi.reshape(-1)),
            dstloc=np.ascontiguousarray(dstloc.T),
            degedge=np.ascontiguousarray(degedge.T),
            degown=np.ascontiguousarray(degown.reshape(NB * 128)
                                        .reshape(NB, 128).T),  # [128, NB]
            bsc=np.ascontiguousarray(bsc.transpose(1, 0, 2)),  # [128, NB, B]
            mine=mine,
        ))
    return dict(plans=plans, c_lo=c_lo, c_hi=c_hi, nchunk=nchunk,
                core_of=core_of, block_of=block_of, slot_of=slot_of,
                row_of=row_of)


def _build(c_lo, c_hi):
    nchunk = NB * (c_lo + c_hi)
    f32 = mybir.dt.float32
    nc = bacc.Bacc("TRN2", target_bir_lowering=False, debug=False,
                   num_devices=NCORE)

    def di(name, shape, dt=f32):
        return nc.dram_tensor(name, shape, dt, kind="ExternalInput")

    bf16 = mybir.dt.bfloat16
    xt = di("xt", [IN, SHARD])
    idxlo = di("idxlo", [128, NB * c_lo * 8], mybir.dt.int16)
    idxhi = di("idxhi", [128, NB * c_hi * 8], mybir.dt.int16)
    dstloc = di("dstloc", [128, nchunk])
    degedge = di("degedge", [128, nchunk])
    degown = di("degown", [128, NB])
    iota = di("iota", [128, 128], bf16)
    slotid = di("slotid", [128, 1])
    ident = di("ident", [128, 128])
    bsc = di("bsc", [128, NB, B])
    ws = [di(f"w{l}t", [IN if l == 1 else H, H]) for l in (1, 2, 3)]
    tbs = [di(f"tb{l}", [H, 1]) for l in (1, 2, 3)]
    hw = {}
    for hname, h1 in (("th", 32), ("lh", 32), ("sh", 16)):
        h2 = {"th": NDEF, "lh": 2, "sh": 1}[hname]
        hw[hname] = (di(f"{hname}w1", [H, h1]), di(f"{hname}b1", [h1, 1]),
                     di(f"{hname}w2", [h1, h2]), di(f"{hname}b2", [h2, 1]))
    out = nc.dram_tensor("out", [NDEF + 3, B], f32, kind="ExternalOutput")

    AF = mybir.ActivationFunctionType
    OP = mybir.AluOpType
    # segment sizes in blocks (ragged: ramp up, then steady)
    STEADY = BLK_PER_SEG
    sizes = []
    for sz in (2, 5):
        if sum(sizes) + sz <= NB:
            sizes.append(sz)
    while sum(sizes) < NB:
        sizes.append(min(STEADY, NB - sum(sizes)))
    SEGS = []
    acc = 0
    for sz in sizes:
        SEGS.append((acc, sz))
        acc += sz
    SEG_LO, SEG_HI = max(sz for _, sz in SEGS) * c_lo, max(sz for _, sz in SEGS) * c_hi

    with tile.TileContext(nc) as tc:
        with (
            tc.tile_pool(name="const", bufs=1) as cpool,
            tc.tile_pool(name="g", bufs=3) as gpool,
            tc.tile_pool(name="ht", bufs=2 * NB) as hpool,
            tc.tile_pool(name="work", bufs=12) as wpool,
            tc.tile_pool(name="ms", bufs=NB + 12) as mpool,
            tc.tile_pool(name="psA", bufs=2, space="PSUM") as psA,
            tc.tile_pool(name="psB", bufs=3, space="PSUM") as psB,
            tc.tile_pool(name="psAcc", bufs=1, space="PSUM") as psAcc,
            tc.tile_pool(name="psC", bufs=2, space="PSUM") as psC,
            tc.tile_pool(name="dram", bufs=1, space="DRAM") as dpool,
        ):
            def load(dram_t, shape, dt=f32, tag=None):
                t = cpool.tile(shape, dt, tag=tag or dram_t.name)
                nc.sync.dma_start(t[:], dram_t[:])
                return t

            xt_sb = load(xt, [IN, SHARD])
            ilo_sb = load(idxlo, [128, NB * c_lo * 8], mybir.dt.int16)
            ihi_sb = load(idxhi, [128, NB * c_hi * 8], mybir.dt.int16)
            dl_sb = load(dstloc, [128, nchunk])
            de_sb = load(degedge, [128, nchunk])
            down_sb = load(degown, [128, NB])
            iota_sb = load(iota, [128, 128], bf16)
            sid_sb = load(slotid, [128, 1])
            id_sb = load(ident, [128, 128])
            bsc_sb = load(bsc, [128, NB, B])
            w_sb = [load(w, [IN if l == 1 else H, H]) for l, w in zip((1, 2, 3), ws)]
            tb_sb = [load(t, [H, 1]) for t in tbs]
            hw_sb = {k: tuple(load(t, list(t.shape), tag=f"{k}{i}")
                              for i, t in enumerate(v)) for k, v in hw.items()}

            # dis = 1/sqrt(deg+1)
            dise = cpool.tile([128, nchunk], f32, tag="dise")
            nc.vector.reciprocal(dise[:], de_sb[:])
            nc.scalar.activation(dise[:], dise[:], AF.Sqrt)

            diso = cpool.tile([128, NB], f32, tag="diso")
            nc.vector.reciprocal(diso[:], down_sb[:])
            nc.scalar.activation(diso[:], diso[:], AF.Sqrt)

            bounce = dpool.tile([SHARD, 2 * H], bf16, tag="bounce")
            msA = dpool.tile([NCORE * SHARD, 2 * H], bf16, tag="msA")
            msB = dpool.tile([NCORE * SHARD, 2 * H], bf16, tag="msB")
            poolin = dpool.tile([H, B], f32, tag="poolin")
            poolout = dpool.tile([H, B], f32, tag="poolout")

            hT = [None] * NB
            msT = [None] * NB
            for l in (1, 2, 3):
                msfull = (msA, msB, msA)[l - 1]
                # PRE: ms shard = dis * (h @ W~), node-major -> bounce
                for b in range(NB):
                    lhsT = xt_sb[:, b * 128:(b + 1) * 128] if l == 1 else hT[b][:]
                    ps = psA.tile([128, H], f32, tag="pre")
                    nc.tensor.matmul(ps[:], lhsT, w_sb[l - 1][:],
                                     start=True, stop=True)
                    ms = mpool.tile([128, H], bf16, tag="ms")
                    nc.scalar.activation(ms[:], ps[:], AF.Copy,
                                         scale=diso[:, b:b + 1])
                    nc.sync.dma_start(bounce[b * 128:(b + 1) * 128, 0:H], ms[:])
                    msT[b] = ms
                if os.environ.get("K_NO_COLL") == "1":
                    for _r in range(NCORE):
                        nc.sync.dma_start(
                            msfull[_r * SHARD:(_r + 1) * SHARD, :], bounce[:])
                else:
                    nc.gpsimd.collective_compute(
                        "AllGather", OP.bypass,
                        ins=[bounce[:].opt()], outs=[msfull[:].opt()],
                        replica_groups=[list(range(NCORE))])
                # MP: gather + scaled-onehot scatter matmuls.
                # Ragged segments: small first gathers shorten the
                # post-AllGather pipeline bubble.
                for s0, scnt in SEGS:
                    glo = gpool.tile([128, SEG_LO, 2 * H], bf16, tag="glo")
                    ghi = gpool.tile([128, SEG_HI, 2 * H], bf16, tag="ghi")
                    if os.environ.get("K_NO_GATHER") == "1":
                        nc.vector.memset(glo[:], 0.0)
                        nc.vector.memset(ghi[:], 0.0)
                    else:
                        nc.gpsimd.dma_gather(
                            glo[:, 0:scnt * c_lo, :], msfull[0:LOROWS, :],
                            ilo_sb[:, s0 * c_lo * 8:(s0 + scnt) * c_lo * 8],
                            scnt * c_lo * 128, scnt * c_lo * 128, 2 * H,
                            single_packet=False)
                        nc.gpsimd.dma_gather(
                            ghi[:, 0:scnt * c_hi, :], msfull[LOROWS:2 * LOROWS, :],
                            ihi_sb[:, s0 * c_hi * 8:(s0 + scnt) * c_hi * 8],
                            scnt * c_hi * 128, scnt * c_hi * 128, 2 * H,
                            single_packet=False)
                    for bb in range(scnt):
                        b = s0 + bb
                        ps = psB.tile([H, 128], f32, tag="mp")
                        for c in range(c_lo + c_hi):
                            gc = b * (c_lo + c_hi) + c
                            g = (glo[:, bb * c_lo + c, 0:H] if c < c_lo
                                 else ghi[:, bb * c_hi + (c - c_lo), 0:H])
                            shot = wpool.tile([128, 128], bf16, tag="shot")
                            nc.vector.tensor_scalar(
                                shot[:], iota_sb[:],
                                dl_sb[:, gc:gc + 1], dise[:, gc:gc + 1],
                                op0=OP.is_equal, op1=OP.mult)
                            nc.tensor.matmul(ps[:], g, shot[:],
                                             start=(c == 0), stop=False)
                        # self-loop term dis_d * ms_d via diagonal scaled-shot
                        dshot = wpool.tile([128, 128], bf16, tag="shot")
                        nc.vector.tensor_scalar(
                            dshot[:], iota_sb[:],
                            sid_sb[:, 0:1], diso[:, b:b + 1],
                            op0=OP.is_equal, op1=OP.mult)
                        nc.tensor.matmul(ps[:], msT[b][:], dshot[:],
                                         start=False, stop=True)
                        h = hpool.tile([H, 128], f32, tag="hT")
                        nc.scalar.activation(h[:], ps[:], AF.Relu,
                                             bias=tb_sb[l - 1][:, 0:1])
                        hT[b] = h

            # Pooling: gembT = sum_b h3_b^T-free... pooledT[f,g] via transpose
            poolps = psAcc.tile([H, B], f32, tag="poolacc")
            for b in range(NB):
                pst = psC.tile([128, H], f32, tag="scratch")
                nc.tensor.transpose(pst[:], hT[b][:], id_sb[0:H, 0:H])
                h3 = wpool.tile([128, H], f32, tag="h3")
                nc.scalar.activation(h3[:], pst[:], AF.Copy)
                nc.tensor.matmul(poolps[:], h3[:], bsc_sb[:, b, :],
                                 start=(b == 0), stop=(b == NB - 1))
            psb = wpool.tile([H, B], f32, tag="poolsb")
            nc.scalar.activation(psb[:], poolps[:], AF.Copy)
            nc.sync.dma_start(poolin[:], psb[:])
            if os.environ.get("K_NO_COLL") == "1":
                nc.sync.dma_start(poolout[:], poolin[:])
            else:
                nc.gpsimd.collective_compute(
                    "AllReduce", OP.add,
                    ins=[poolin[:].opt()], outs=[poolout[:].opt()],
                    replica_groups=[list(range(NCORE))])
            gemb = wpool.tile([H, B], f32, tag="gemb")
            nc.sync.dma_start(gemb[:], poolout[:])

            # Heads (computed replicated on every core)
            for hname, r0, act in (("th", 0, None), ("lh", NDEF, AF.Sigmoid),
                                   ("sh", NDEF + 2, AF.Sigmoid)):
                w1, b1, w2, b2 = hw_sb[hname]
                h1 = w1.shape[1]
                h2 = w2.shape[1]
                p1t = psC.tile([128, B], f32, tag="scratch")
                p1 = p1t[0:h1, :]
                nc.tensor.matmul(p1, w1[:], gemb[:], start=True, stop=True)
                a1 = wpool.tile([h1, B], f32, tag="hd1sb")
                nc.scalar.activation(a1[:], p1, AF.Relu, bias=b1[:, 0:1])
                p2t = psC.tile([128, B], f32, tag="scratch")
                p2 = p2t[0:h2, :]
                nc.tensor.matmul(p2, w2[:], a1[:], start=True, stop=True)
                hsb = wpool.tile([h2, B], f32, tag="hdout")
                if act is None:
                    nc.vector.tensor_scalar_add(hsb[:], p2, b2[:, 0:1])
                else:
                    nc.scalar.activation(hsb[:], p2, act, bias=b2[:, 0:1])
                nc.sync.dma_start(out[r0:r0 + h2, :], hsb[:])

    nc.compile()
    return nc


def prepare(x, edge_index, batch,
            W1, b1, W2, b2, W3, b3,
            bn1_g, bn1_b, bn1_m, bn1_v,
            bn2_g, bn2_b, bn2_m, bn2_v,
            bn3_g, bn3_b, bn3_m, bn3_v,
            th_W1, th_b1, th_W2, th_b2,
            lh_W1, lh_b1, lh_W2, lh_b2,
            sh_W1, sh_b1, sh_W2, sh_b2):
    x = np.asarray(x, np.float32)
    edge_index = np.asarray(edge_index)
    batch = np.asarray(batch)
    src, dst = np.asarray(edge_index[0], np.int64), np.asarray(edge_index[1], np.int64)

    plan = _plan(src, dst, np.asarray(batch, np.int64))
    c_lo, c_hi = plan["c_lo"], plan["c_hi"]

    key = (c_lo, c_hi)
    _last_cfg[0] = key
    if key not in _cache:
        _cache[key] = _build(c_lo, c_hi)
    nc = _cache[key]

    # BN-folded weights
    def fold(W, bb, g, beta, mu, v):
        s = np.asarray(g) / np.sqrt(np.asarray(v) + EPS)
        Wt = np.asarray(W, np.float32) * s[None, :]
        tb = ((np.asarray(bb) - np.asarray(mu)) * s + np.asarray(beta))
        return Wt.astype(np.float32), tb.astype(np.float32).reshape(H, 1)

    w1t, tb1 = fold(W1, b1, bn1_g, bn1_b, bn1_m, bn1_v)
    w2t, tb2 = fold(W2, b2, bn2_g, bn2_b, bn2_m, bn2_v)
    w3t, tb3 = fold(W3, b3, bn3_g, bn3_b, bn3_m, bn3_v)

    import ml_dtypes
    iota_np = np.tile(np.arange(128, dtype=np.float32), (128, 1)).astype(ml_dtypes.bfloat16)
    ident_np = np.eye(128, dtype=np.float32)

    in_maps = []
    for c in range(NCORE):
        p = plan["plans"][c]
        mine = p["mine"]
        xts = np.zeros((IN, SHARD), np.float32)
        cols = plan["block_of"][mine] * 128 + plan["slot_of"][mine]
        xts[:, cols] = x[mine].T
        in_maps.append({
            "xt": xts, "idxlo": p["idxlo"], "idxhi": p["idxhi"],
            "slotid": np.arange(128, dtype=np.float32).reshape(128, 1),
            "dstloc": p["dstloc"], "degedge": p["degedge"],
            "degown": p["degown"], "iota": iota_np, "ident": ident_np,
            "bsc": p["bsc"],
            "w1t": w1t, "w2t": w2t, "w3t": w3t,
            "tb1": tb1, "tb2": tb2, "tb3": tb3,
            "thw1": np.asarray(th_W1, np.float32), "thb1": np.asarray(th_b1, np.float32).reshape(-1, 1),
            "thw2": np.asarray(th_W2, np.float32), "thb2": np.asarray(th_b2, np.float32).reshape(-1, 1),
            "lhw1": np.asarray(lh_W1, np.float32), "lhb1": np.asarray(lh_b1, np.float32).reshape(-1, 1),
            "lhw2": np.asarray(lh_W2, np.float32), "lhb2": np.asarray(lh_b2, np.float32).reshape(-1, 1),
            "shw1": np.asarray(sh_W1, np.float32), "shb1": np.asarray(sh_b1, np.float32).reshape(-1, 1),
            "shw2": np.asarray(sh_W2, np.float32), "shb2": np.asarray(sh_b2, np.float32).reshape(-1, 1),
        })

    return nc, in_maps


def kernel(**inputs):
    nc, in_maps = prepare(**inputs)
    kernel._last_clo, kernel._last_chi = _last_cfg[0]
    res = bass_utils.run_bass_kernel_spmd(nc, in_maps, core_ids=list(range(NCORE)))
    kernel._last_results = res
    o = res.results[0]["out"]  # [9, B]
    type_logits = np.ascontiguousarray(o[0:NDEF].T)
    location = np.ascontiguousarray(o[NDEF:NDEF + 2].T)
    severity = np.ascontiguousarray(o[NDEF + 2:NDEF + 3].T)
    return (type_logits, location, severity)


# revision 35
# speedup vs baseline: 1.0046x; 1.0020x over previous
import os
import sys
import numpy as np

try:
    import concourse  # noqa: F401
except ImportError:
    for _p in ("/opt/trn_rl_repo", "/root/.axon_site/_ro/trn_rl_repo"):
        if os.path.isdir(_p):
            sys.path.insert(0, _p)
            break

from concourse import bass, bacc, tile, mybir, bass_utils  # noqa: E402

# Problem constants (nn_DefectPredictionGNN: 3-layer GCN + mean-pool + 3 heads)
N, E, B = 50000, 800000, 64
IN, H = 3, 64
NDEF = 6
EPS = 1e-5
NCORE = 8
NB = 52                      # dst blocks per core
SHARD = NB * 128             # 6656 padded slots per core
LOROWS = 4 * SHARD           # 26624 (< int16 range)
BLK_PER_SEG = 5              # blocks per gather segment
NSEG = NB // BLK_PER_SEG     # 10

_cache = {}
_last_cfg = [(9, 9)]


def _plan(src, dst, batch):
    """Host-side index preprocessing: node->core/block/slot assignment,
    per-core gather lists and chunk metadata. Pure integer work."""
    deg = np.bincount(dst, minlength=N).astype(np.int64)   # in-degree (no self)
    degp1 = deg + 1

    # Stage 1: assign nodes to cores, snake over degree-sorted order.
    order = np.argsort(-degp1, kind="stable")
    core_of = np.empty(N, np.int32)
    snake = np.concatenate([np.arange(NCORE), np.arange(NCORE)[::-1]])
    core_of[order] = snake[np.arange(N) % (2 * NCORE)]

    # Self-loops are handled on-device via a diagonal scaled-shot matmul
    # (ms is already resident in SBUF) -- they do not enter the gather lists.
    if os.environ.get("K_SELF_EDGES") == "1":
        src_all = np.concatenate([src, np.arange(N, dtype=src.dtype)])
        dst_all = np.concatenate([dst, np.arange(N, dtype=dst.dtype)])
    else:
        src_all, dst_all = src, dst

    # lo/hi membership of a SOURCE node = its core's half (cores 0-3 are rows
    # [0, LOROWS)). Known after stage 1, independent of block assignment.
    lodeg = np.bincount(dst_all[core_of[src_all] < 4], minlength=N)
    hideg = np.bincount(dst_all[core_of[src_all] >= 4], minlength=N)

    # Stage 2: per core, greedily pack nodes into 49 blocks (<=128 nodes each)
    # balancing (lo, hi) in-edge loads.
    block_of = np.empty(N, np.int32)
    slot_of = np.empty(N, np.int32)
    for c in range(NCORE):
        nodes = np.where(core_of == c)[0]
        nodes = nodes[np.argsort(-(lodeg[nodes] + hideg[nodes]), kind="stable")]
        loads_lo = np.zeros(NB, np.int64)
        loads_hi = np.zeros(NB, np.int64)
        counts = np.zeros(NB, np.int64)
        for u in nodes:
            lv, hv = lodeg[u], hideg[u]
            cand = np.maximum(loads_lo + lv, loads_hi + hv).astype(np.float64)
            cand[counts >= 128] = np.inf
            j = int(np.argmin(cand))
            block_of[u] = j
            slot_of[u] = counts[j]
            counts[j] += 1
            loads_lo[j] += lv
            loads_hi[j] += hv

    # Global HBM row of each node: (core, block, slot)
    row_of = (core_of.astype(np.int64) * SHARD + block_of * 128 + slot_of)

    # Edge placement
    e_core = core_of[dst_all]
    e_block = block_of[dst_all]
    e_slot = slot_of[dst_all]
    e_srow = row_of[src_all]
    e_islo = e_srow < LOROWS
    e_deg = degp1[dst_all]

    # Chunk capacity per (core, block, half)
    max_lo = max_hi = 0
    per = {}
    for c in range(NCORE):
        mc = e_core == c
        for half, mh in (("lo", e_islo), ("hi", ~e_islo)):
            m = mc & mh
            cnt = np.bincount(e_block[m], minlength=NB)
            per[(c, half)] = m
            if half == "lo":
                max_lo = max(max_lo, int(cnt.max()))
            else:
                max_hi = max(max_hi, int(cnt.max()))
    c_lo = max(1, -(-max_lo // 128))
    c_hi = max(1, -(-max_hi // 128))
    nchunk = NB * (c_lo + c_hi)

    plans = []
    for c in range(NCORE):
        idx_lo = np.zeros((NB, c_lo * 128), np.int16)
        idx_hi = np.zeros((NB, c_hi * 128), np.int16)
        dstloc = np.full((nchunk, 128), -1.0, np.float32)
        degedge = np.ones((nchunk, 128), np.float32)
        for half, idx_arr, cc, off in (("lo", idx_lo, c_lo, 0), ("hi", idx_hi, c_hi, c_lo)):
            m = (e_core == c) & (e_islo if half == "lo" else ~e_islo)
            eb, es, er, ed = e_block[m], e_slot[m], e_srow[m], e_deg[m]
            if half == "hi":
                er = er - LOROWS
            o = np.argsort(eb, kind="stable")
            eb, es, er, ed = eb[o], es[o], er[o], ed[o]
            starts = np.searchsorted(eb, np.arange(NB))
            ends = np.searchsorted(eb, np.arange(NB) + 1)
            for b in range(NB):
                k = ends[b] - starts[b]
                sl = slice(starts[b], ends[b])
                idx_arr[b, :k] = er[sl].astype(np.int16)
                gc0 = b * (c_lo + c_hi) + off
                dl = dstloc[gc0:gc0 + cc].reshape(-1)
                dl[:k] = es[sl]
                de = degedge[gc0:gc0 + cc].reshape(-1)
                de[:k] = ed[sl]

        def wrap(a):  # [L] int16 -> [128, L//16] wrapped + replicated
            L = a.size
            w = a.reshape(L // 16, 16).T  # [16, L//16]
            return np.tile(w, (8, 1)).copy()

        degown = np.ones((NB, 128), np.float32)
        mine = np.where(core_of == c)[0]
        degown[block_of[mine], slot_of[mine]] = degp1[mine]

        bsc = np.zeros((NB, 128, B), np.float32)
        cnt = np.bincount(batch, minlength=B).astype(np.float32)
        invcnt = 1.0 / np.maximum(cnt, 1.0)
        bsc[block_of[mine], slot_of[mine], batch[mine]] = invcnt[batch[mine]]

        plans.append(dict(
            idxlo=wrap(idx_lo.reshape(-1)),
            idxhi=wrap(idx_hi.reshape(-1)),
            dstloc=np.ascontiguousarray(dstloc.T),
            degedge=np.ascontiguousarray(degedge.T),
            degown=np.ascontiguousarray(degown.reshape(NB * 128)
                                        .reshape(NB, 128).T),  # [128, NB]
            bsc=np.ascontiguousarray(bsc.transpose(1, 0, 2)),  # [128, NB, B]
            mine=mine,
        ))
    return dict(plans=plans, c_lo=c_lo, c_hi=c_hi, nchunk=nchunk,
                core_of=core_of, block_of=block_of, slot_of=slot_of,
                row_of=row_of)


def _build(c_lo, c_hi):
    nchunk = NB * (c_lo + c_hi)
    f32 = mybir.dt.float32
    nc = bacc.Bacc("TRN2", target_bir_lowering=False, debug=False,
                   num_devices=NCORE)

    def di(name, shape, dt=f32):
        return nc.dram_tensor(name, shape, dt, kind="ExternalInput")

    bf16 = mybir.dt.bfloat16
    xt = di("xt", [IN, SHARD])
    idxlo = di("idxlo", [128, NB * c_lo * 8], mybir.dt.int16)
    idxhi = di("idxhi", [128, NB * c_hi * 8], mybir.dt.int16)
    dstloc = di("dstloc", [128, nchunk])
    degedge = di("degedge", [128, nchunk])
    degown = di("degown", [128, NB])
    iota = di("iota", [128, 128], bf16)
    slotid = di("slotid", [128, 1])
    ident = di("ident", [128, 128])
    bsc = di("bsc", [128, NB, B])
    ws = [di(f"w{l}t", [IN if l == 1 else H, H]) for l in (1, 2, 3)]
    tbs = [di(f"tb{l}", [H, 1]) for l in (1, 2, 3)]
    hw = {}
    for hname, h1 in (("th", 32), ("lh", 32), ("sh", 16)):
        h2 = {"th": NDEF, "lh": 2, "sh": 1}[hname]
        hw[hname] = (di(f"{hname}w1", [H, h1]), di(f"{hname}b1", [h1, 1]),
                     di(f"{hname}w2", [h1, h2]), di(f"{hname}b2", [h2, 1]))
    out = nc.dram_tensor("out", [NDEF + 3, B], f32, kind="ExternalOutput")

    AF = mybir.ActivationFunctionType
    OP = mybir.AluOpType
    # segment sizes in blocks (ragged: ramp up, then steady)
    STEADY = BLK_PER_SEG
    sizes = []
    for sz in ():
        if sum(sizes) + sz <= NB:
            sizes.append(sz)
    while sum(sizes) < NB:
        sizes.append(min(STEADY, NB - sum(sizes)))
    SEGS = []
    acc = 0
    for sz in sizes:
        SEGS.append((acc, sz))
        acc += sz
    SEG_LO, SEG_HI = max(sz for _, sz in SEGS) * c_lo, max(sz for _, sz in SEGS) * c_hi

    with tile.TileContext(nc) as tc:
        with (
            tc.tile_pool(name="const", bufs=1) as cpool,
            tc.tile_pool(name="g", bufs=3) as gpool,
            tc.tile_pool(name="ht", bufs=2 * NB) as hpool,
            tc.tile_pool(name="work", bufs=12) as wpool,
            tc.tile_pool(name="ms", bufs=NB + 12) as mpool,
            tc.tile_pool(name="psA", bufs=2, space="PSUM") as psA,
            tc.tile_pool(name="psB", bufs=3, space="PSUM") as psB,
            tc.tile_pool(name="psAcc", bufs=1, space="PSUM") as psAcc,
            tc.tile_pool(name="psC", bufs=2, space="PSUM") as psC,
            tc.tile_pool(name="dram", bufs=1, space="DRAM") as dpool,
        ):
            def load(dram_t, shape, dt=f32, tag=None):
                t = cpool.tile(shape, dt, tag=tag or dram_t.name)
                nc.sync.dma_start(t[:], dram_t[:])
                return t

            xt_sb = load(xt, [IN, SHARD])
            ilo_sb = load(idxlo, [128, NB * c_lo * 8], mybir.dt.int16)
            ihi_sb = load(idxhi, [128, NB * c_hi * 8], mybir.dt.int16)
            dl_sb = load(dstloc, [128, nchunk])
            de_sb = load(degedge, [128, nchunk])
            down_sb = load(degown, [128, NB])
            iota_sb = load(iota, [128, 128], bf16)
            sid_sb = load(slotid, [128, 1])
            id_sb = load(ident, [128, 128])
            bsc_sb = load(bsc, [128, NB, B])
            w_sb = [load(w, [IN if l == 1 else H, H]) for l, w in zip((1, 2, 3), ws)]
            tb_sb = [load(t, [H, 1]) for t in tbs]
            hw_sb = {k: tuple(load(t, list(t.shape), tag=f"{k}{i}")
                              for i, t in enumerate(v)) for k, v in hw.items()}

            # dis = 1/sqrt(deg+1)
            dise = cpool.tile([128, nchunk], f32, tag="dise")
            nc.vector.reciprocal(dise[:], de_sb[:])
            nc.scalar.activation(dise[:], dise[:], AF.Sqrt)

            diso = cpool.tile([128, NB], f32, tag="diso")
            nc.vector.reciprocal(diso[:], down_sb[:])
            nc.scalar.activation(diso[:], diso[:], AF.Sqrt)

            bounce = dpool.tile([SHARD, 2 * H], bf16, tag="bounce")
            msA = dpool.tile([NCORE * SHARD, 2 * H], bf16, tag="msA")
            msB = dpool.tile([NCORE * SHARD, 2 * H], bf16, tag="msB")
            poolin = dpool.tile([H, B], f32, tag="poolin")
            poolout = dpool.tile([H, B], f32, tag="poolout")

            hT = [None] * NB
            msT = [None] * NB
            for l in (1, 2, 3):
                msfull = (msA, msB, msA)[l - 1]
                # PRE: ms shard = dis * (h @ W~), node-major -> bounce
                for b in range(NB):
                    lhsT = xt_sb[:, b * 128:(b + 1) * 128] if l == 1 else hT[b][:]
                    ps = psA.tile([128, H], f32, tag="pre")
                    nc.tensor.matmul(ps[:], lhsT, w_sb[l - 1][:],
                                     start=True, stop=True)
                    ms = mpool.tile([128, H], bf16, tag="ms")
                    nc.scalar.activation(ms[:], ps[:], AF.Copy,
                                         scale=diso[:, b:b + 1])
                    nc.sync.dma_start(bounce[b * 128:(b + 1) * 128, 0:H], ms[:])
                    msT[b] = ms
                if os.environ.get("K_NO_COLL") == "1":
                    for _r in range(NCORE):
                        nc.sync.dma_start(
                            msfull[_r * SHARD:(_r + 1) * SHARD, :], bounce[:])
                else:
                    nc.gpsimd.collective_compute(
                        "AllGather", OP.bypass,
                        ins=[bounce[:].opt()], outs=[msfull[:].opt()],
                        replica_groups=[list(range(NCORE))])
                # MP: gather + scaled-onehot scatter matmuls.
                # Ragged segments: small first gathers shorten the
                # post-AllGather pipeline bubble.
                for s0, scnt in SEGS:
                    glo = gpool.tile([128, SEG_LO, 2 * H], bf16, tag="glo")
                    ghi = gpool.tile([128, SEG_HI, 2 * H], bf16, tag="ghi")
                    if os.environ.get("K_NO_GATHER") == "1":
                        nc.vector.memset(glo[:], 0.0)
                        nc.vector.memset(ghi[:], 0.0)
                    else:
                        nc.gpsimd.dma_gather(
                            glo[:, 0:scnt * c_lo, :], msfull[0:LOROWS, :],
                            ilo_sb[:, s0 * c_lo * 8:(s0 + scnt) * c_lo * 8],
                            scnt * c_lo * 128, scnt * c_lo * 128, 2 * H,
                            single_packet=False)
                        nc.gpsimd.dma_gather(
                            ghi[:, 0:scnt * c_hi, :], msfull[LOROWS:2 * LOROWS, :],
                            ihi_sb[:, s0 * c_hi * 8:(s0 + scnt) * c_hi * 8],
                            scnt * c_hi * 128, scnt * c_hi * 128, 2 * H,
                            single_packet=False)
                    for bb in range(scnt):
                        b = s0 + bb
                        ps = psB.tile([H, 128], f32, tag="mp")
                        for c in range(c_lo + c_hi):
                            gc = b * (c_lo + c_hi) + c
                            g = (glo[:, bb * c_lo + c, 0:H] if c < c_lo
                                 else ghi[:, bb * c_hi + (c - c_lo), 0:H])
                            shot = wpool.tile([128, 128], bf16, tag="shot")
                            nc.vector.tensor_scalar(
                                shot[:], iota_sb[:],
                                dl_sb[:, gc:gc + 1], dise[:, gc:gc + 1],
                                op0=OP.is_equal, op1=OP.mult)
                            nc.tensor.matmul(ps[:], g, shot[:],
                                             start=(c == 0), stop=False)
                        # self-loop term dis_d * ms_d via diagonal scaled-shot
                        dshot = wpool.tile([128, 128], bf16, tag="shot")
                        nc.vector.tensor_scalar(
                            dshot[:], iota_sb[:],
                            sid_sb[:, 0:1], diso[:, b:b + 1],
                            op0=OP.is_equal, op1=OP.mult)
                        nc.tensor.matmul(ps[:], msT[b][:], dshot[:],
                                         start=False, stop=True)
                        h = hpool.tile([H, 128], f32, tag="hT")
                        nc.scalar.activation(h[:], ps[:], AF.Relu,
                                             bias=tb_sb[l - 1][:, 0:1])
                        hT[b] = h

            # Pooling: gembT = sum_b h3_b^T-free... pooledT[f,g] via transpose
            poolps = psAcc.tile([H, B], f32, tag="poolacc")
            for b in range(NB):
                pst = psC.tile([128, H], f32, tag="scratch")
                nc.tensor.transpose(pst[:], hT[b][:], id_sb[0:H, 0:H])
                h3 = wpool.tile([128, H], f32, tag="h3")
                nc.scalar.activation(h3[:], pst[:], AF.Copy)
                nc.tensor.matmul(poolps[:], h3[:], bsc_sb[:, b, :],
                                 start=(b == 0), stop=(b == NB - 1))
            psb = wpool.tile([H, B], f32, tag="poolsb")
            nc.scalar.activation(psb[:], poolps[:], AF.Copy)
            nc.sync.dma_start(poolin[:], psb[:])
            if os.environ.get("K_NO_COLL") == "1":
                nc.sync.dma_start(poolout[:], poolin[:])
            else:
                nc.gpsimd.collective_compute(
                    "AllReduce", OP.add,
                    ins=[poolin[:].opt()], outs=[poolout[:].opt()],
                    replica_groups=[list(range(NCORE))])
            gemb = wpool.tile([H, B], f32, tag="gemb")
            nc.sync.dma_start(gemb[:], poolout[:])

            # Heads (computed replicated on every core)
            for hname, r0, act in (("th", 0, None), ("lh", NDEF, AF.Sigmoid),
                                   ("sh", NDEF + 2, AF.Sigmoid)):
                w1, b1, w2, b2 = hw_sb[hname]
                h1 = w1.shape[1]
                h2 = w2.shape[1]
                p1t = psC.tile([128, B], f32, tag="scratch")
                p1 = p1t[0:h1, :]
                nc.tensor.matmul(p1, w1[:], gemb[:], start=True, stop=True)
                a1 = wpool.tile([h1, B], f32, tag="hd1sb")
                nc.scalar.activation(a1[:], p1, AF.Relu, bias=b1[:, 0:1])
                p2t = psC.tile([128, B], f32, tag="scratch")
                p2 = p2t[0:h2, :]
                nc.tensor.matmul(p2, w2[:], a1[:], start=True, stop=True)
                hsb = wpool.tile([h2, B], f32, tag="hdout")
                if act is None:
                    nc.vector.tensor_scalar_add(hsb[:], p2, b2[:, 0:1])
                else:
                    nc.scalar.activation(hsb[:], p2, act, bias=b2[:, 0:1])
                nc.sync.dma_start(out[r0:r0 + h2, :], hsb[:])

    nc.compile()
    return nc


def prepare(x, edge_index, batch,
            W1, b1, W2, b2, W3, b3,
            bn1_g, bn1_b, bn1_m, bn1_v,
            bn2_g, bn2_b, bn2_m, bn2_v,
            bn3_g, bn3_b, bn3_m, bn3_v,
            th_W1, th_b1, th_W2, th_b2,
            lh_W1, lh_b1, lh_W2, lh_b2,
            sh_W1, sh_b1, sh_W2, sh_b2):
    x = np.asarray(x, np.float32)
    edge_index = np.asarray(edge_index)
    batch = np.asarray(batch)
    src, dst = np.asarray(edge_index[0], np.int64), np.asarray(edge_index[1], np.int64)

    plan = _plan(src, dst, np.asarray(batch, np.int64))
    c_lo, c_hi = plan["c_lo"], plan["c_hi"]

    key = (c_lo, c_hi)
    _last_cfg[0] = key
    if key not in _cache:
        _cache[key] = _build(c_lo, c_hi)
    nc = _cache[key]

    # BN-folded weights
    def fold(W, bb, g, beta, mu, v):
        s = np.asarray(g) / np.sqrt(np.asarray(v) + EPS)
        Wt = np.asarray(W, np.float32) * s[None, :]
        tb = ((np.asarray(bb) - np.asarray(mu)) * s + np.asarray(beta))
        return Wt.astype(np.float32), tb.astype(np.float32).reshape(H, 1)

    w1t, tb1 = fold(W1, b1, bn1_g, bn1_b, bn1_m, bn1_v)
    w2t, tb2 = fold(W2, b2, bn2_g, bn2_b, bn2_m, bn2_v)
    w3t, tb3 = fold(W3, b3, bn3_g, bn3_b, bn3_m, bn3_v)

    import ml_dtypes
    iota_np = np.tile(np.arange(128, dtype=np.float32), (128, 1)).astype(ml_dtypes.bfloat16)
    ident_np = np.eye(128, dtype=np.float32)

    in_maps = []
    for c in range(NCORE):
        p = plan["plans"][c]
        mine = p["mine"]
        xts = np.zeros((IN, SHARD), np.float32)
        cols = plan["block_of"][mine] * 128 + plan["slot_of"][mine]
        xts[:, cols] = x[mine].T
        in_maps.append({
            "xt": xts, "idxlo": p["idxlo"], "idxhi": p["idxhi"],
            "slotid": np.arange(128, dtype=np.float32).reshape(128, 1),
            "dstloc": p["dstloc"], "degedge": p["degedge"],
            "degown": p["degown"], "iota": iota_np, "ident": ident_np,
            "bsc": p["bsc"],
            "w1t": w1t, "w2t": w2t, "w3t": w3t,
            "tb1": tb1, "tb2": tb2, "tb3": tb3,
            "thw1": np.asarray(th_W1, np.float32), "thb1": np.asarray(th_b1, np.float32).reshape(-1, 1),
            "thw2": np.asarray(th_W2, np.float32), "thb2": np.asarray(th_b2, np.float32).reshape(-1, 1),
            "lhw1": np.asarray(lh_W1, np.float32), "lhb1": np.asarray(lh_b1, np.float32).reshape(-1, 1),
            "lhw2": np.asarray(lh_W2, np.float32), "lhb2": np.asarray(lh_b2, np.float32).reshape(-1, 1),
            "shw1": np.asarray(sh_W1, np.float32), "shb1": np.asarray(sh_b1, np.float32).reshape(-1, 1),
            "shw2": np.asarray(sh_W2, np.float32), "shb2": np.asarray(sh_b2, np.float32).reshape(-1, 1),
        })

    return nc, in_maps


def kernel(**inputs):
    nc, in_maps = prepare(**inputs)
    kernel._last_clo, kernel._last_chi = _last_cfg[0]
    res = bass_utils.run_bass_kernel_spmd(nc, in_maps, core_ids=list(range(NCORE)))
    kernel._last_results = res
    o = res.results[0]["out"]  # [9, B]
    type_logits = np.ascontiguousarray(o[0:NDEF].T)
    location = np.ascontiguousarray(o[NDEF:NDEF + 2].T)
    severity = np.ascontiguousarray(o[NDEF + 2:NDEF + 3].T)
    return (type_logits, location, severity)
